# revision 8
# baseline (speedup 1.0000x reference)
import numpy as np

# Problem constants (hardcoded per contract)
B, N, P, L = 16, 300, 100, 32
H = W = 768
NCORES = 8
BPC = B // NCORES  # batches per core = 2
CONF_TH, MIN_IOU, OCR_TH = 0.3, 0.5, 0.7
DS_W, PD_W, OP_W = 0.1, 0.1, 0.1

_nc = None


def _build():
    import os
    STAGES = os.environ.get("KSTAGES", "ABCDE")
    global _nc
    if _nc is not None:
        return _nc
    import concourse.bacc as bacc
    import concourse.tile as tile
    from concourse import mybir

    f32 = mybir.dt.float32
    f16 = mybir.dt.float16
    i32 = mybir.dt.int32
    Alu = mybir.AluOpType
    Act = mybir.ActivationFunctionType
    AX = mybir.AxisListType.X

    nc = bacc.Bacc("TRN2", target_bir_lowering=False, debug=False,
                   num_devices=NCORES)

    seg = nc.dram_tensor("seg_ch", [BPC, 2, H, W], f32, kind="ExternalInput")
    dbx = nc.dram_tensor("det_boxes", [BPC, N, 4], f32, kind="ExternalInput")
    dlg = nc.dram_tensor("det_logits", [BPC, N, 7], f32, kind="ExternalInput")
    pbx = nc.dram_tensor("plate_boxes", [BPC, P, 4], f32, kind="ExternalInput")
    pcf = nc.dram_tensor("plate_conf", [BPC, P], f32, kind="ExternalInput")
    ocr = nc.dram_tensor("ocr", [BPC, L, 37], f32, kind="ExternalInput")
    out = nc.dram_tensor("partials", [1, 4], f32, kind="ExternalOutput")
    # DRAM scratch for row round-trips (per batch)
    scr32 = nc.dram_tensor("scr32", [BPC, 5, N], f32, kind="Internal")
    scr16a = nc.dram_tensor("scr16a", [BPC, N], f16, kind="Internal")
    scr16b = nc.dram_tensor("scr16b", [BPC, N], f16, kind="Internal")

    NCH = [(0, 128), (128, 256), (256, 300)]  # n-chunks
    XT = 6  # y tiles of 128

    with tile.TileContext(nc) as tc:
        import contextlib
        ctx = contextlib.ExitStack()
        with ctx:
            const = ctx.enter_context(tc.tile_pool(name="const", bufs=1))
            small = ctx.enter_context(tc.tile_pool(name="small", bufs=3))
            chp = ctx.enter_context(tc.tile_pool(name="chp", bufs=4))
            dp = ctx.enter_context(tc.tile_pool(name="dp", bufs=1))
            atp = ctx.enter_context(tc.tile_pool(name="atp", bufs=1))
            bcp = ctx.enter_context(tc.tile_pool(name="bcp", bufs=2))
            gp = ctx.enter_context(tc.tile_pool(name="gp", bufs=2))
            psp = ctx.enter_context(tc.tile_pool(name="psp", bufs=2, space="PSUM"))
            psr = ctx.enter_context(tc.tile_pool(name="psr", bufs=1, space="PSUM"))

            # ---- shared constants ----
            iox_i = const.tile([128, W], i32)
            nc.gpsimd.iota(iox_i[:], pattern=[[1, W]], base=0, channel_multiplier=0)
            iox16 = const.tile([128, W], f16)
            nc.vector.tensor_copy(iox16[:], iox_i[:])
            ioy_i = const.tile([128, XT], i32)
            nc.gpsimd.iota(ioy_i[:], pattern=[[128, XT]], base=0, channel_multiplier=1)
            ioy = const.tile([128, XT], f32)
            nc.vector.tensor_copy(ioy[:], ioy_i[:])
            ones = const.tile([128, 1], f32)
            nc.vector.memset(ones[:], 1.0)

            # redcol: packed per-partition columns for the final ones-matmul
            # cols 0-5: seg contribs (3 chunks x 2 batches)
            # 6,7: pd term_raw / pvalid (b0); 8,9: (b1)
            # 10,11: ocr max (b0, b1); 12-17: vvalid chunks (b0: 12-14, b1: 15-17)
            RC = 18
            redcol = const.tile([128, RC], f32)
            nc.vector.memset(redcol[:], 0.0)


            def recip(dst, src, pn, tagp):
                # 1/x via exp(-ln x) + 2 Newton steps (nc.vector.reciprocal
                # crashes the exec unit in this runtime; tt-divide is invalid ISA)
                lnv = small.tile([128, dst.shape[1]], f32, tag=tagp + "ln")
                nc.scalar.activation(lnv[:pn], src, Act.Ln, bias=0.0, scale=1.0)
                nc.scalar.activation(dst, lnv[:pn], Act.Exp, bias=0.0, scale=-1.0)
                t1 = small.tile([128, dst.shape[1]], f32, tag=tagp + "t1")
                t2 = small.tile([128, dst.shape[1]], f32, tag=tagp + "t2")
                for _ in range(2):
                    nc.vector.tensor_mul(t1[:pn], src, dst)
                    nc.vector.tensor_scalar(t2[:pn], t1[:pn], -1.0, 2.0, Alu.mult, Alu.add)
                    nc.vector.tensor_mul(dst, dst, t2[:pn])

            # per-batch per-chunk SBUF keeps
            xf_keep = {}   # (b,c) -> [Pn,4] floors (x1f,y1f,x2f,y2f)
            conf_keep = {}

            # ================= Stage A: coords + conf + scratch writes ========
            for b in range(BPC):
                for c, (lo, hi) in enumerate(NCH):
                    Pn = hi - lo
                    bx = small.tile([128, 4], f32, tag="bx")
                    nc.sync.dma_start(bx[:Pn, :], dbx.ap()[b, lo:hi, :])
                    lg = small.tile([128, 7], f32, tag="lg")
                    nc.sync.dma_start(lg[:Pn, :], dlg.ap()[b, lo:hi, :])

                    xyxy = small.tile([128, 4], f32, tag="xyxy")
                    half = small.tile([128, 2], f32, tag="half")
                    nc.vector.tensor_scalar(half[:Pn], bx[:Pn, 2:4], 0.5, None, Alu.mult)
                    nc.vector.tensor_sub(xyxy[:Pn, 0:2], bx[:Pn, 0:2], half[:Pn])
                    nc.vector.tensor_add(xyxy[:Pn, 2:4], bx[:Pn, 0:2], half[:Pn])

                    sc = small.tile([128, 4], f32, tag="sc")
                    nc.vector.tensor_scalar(sc[:Pn], xyxy[:Pn], 768.0, 0.0, Alu.mult, Alu.max)
                    uc = small.tile([128, 4], f32, tag="uc")
                    nc.vector.tensor_scalar(uc[:Pn], sc[:Pn], 767.0, None, Alu.min)
                    ci = small.tile([128, 4], i32, tag="ci")
                    nc.vector.tensor_copy(ci[:Pn], uc[:Pn])
                    cf = small.tile([128, 4], f32, tag="cf")
                    nc.vector.tensor_copy(cf[:Pn], ci[:Pn])
                    gt = small.tile([128, 4], f32, tag="gt")
                    nc.vector.tensor_tensor(gt[:Pn], cf[:Pn], uc[:Pn], Alu.is_gt)
                    flr = small.tile([128, 4], f32, tag=f"flr{b}{c}")
                    nc.vector.tensor_sub(flr[:Pn], cf[:Pn], gt[:Pn])
                    xf_keep[(b, c)] = (flr, Pn)

                    # det conf = 1/sum(exp(x-max))
                    m = small.tile([128, 1], f32, tag="m")
                    nc.vector.tensor_reduce(m[:Pn], lg[:Pn, :], axis=AX, op=Alu.max)
                    negm = small.tile([128, 1], f32, tag="negm")
                    nc.vector.tensor_scalar(negm[:Pn], m[:Pn], -1.0, None, Alu.mult)
                    ex = small.tile([128, 7], f32, tag="ex")
                    nc.scalar.activation(ex[:Pn], lg[:Pn, :], Act.Exp, bias=negm[:Pn], scale=1.0)
                    s = small.tile([128, 1], f32, tag="s")
                    nc.vector.tensor_reduce(s[:Pn], ex[:Pn], axis=AX, op=Alu.add)
                    conf = small.tile([128, 1], f32, tag=f"conf{b}{c}")
                    recip(conf[:Pn], s[:Pn], Pn, "cf")
                    conf_keep[(b, c)] = conf

                    # vvalid (strict >) and poisoned ux1
                    vv = small.tile([128, 1], f32, tag="vv")
                    nc.vector.tensor_scalar(vv[:Pn], conf[:Pn], CONF_TH, None, Alu.is_gt)
                    pois = small.tile([128, 1], f32, tag="pois")
                    nc.vector.tensor_scalar(pois[:Pn], vv[:Pn], -1.0e6, 1.0e6, Alu.mult, Alu.add)
                    ux1p = small.tile([128, 1], f32, tag="ux1p")
                    nc.vector.tensor_add(ux1p[:Pn], xyxy[:Pn, 0:1], pois[:Pn])
                    d2 = small.tile([128, 2], f32, tag="d2")
                    nc.vector.tensor_sub(d2[:Pn], xyxy[:Pn, 2:4], xyxy[:Pn, 0:2])
                    a2 = small.tile([128, 1], f32, tag="a2")
                    nc.vector.tensor_mul(a2[:Pn], d2[:Pn, 0:1], d2[:Pn, 1:2])

                    # vvalid count into redcol (cols 12..17)
                    nc.scalar.copy(redcol[:Pn, 12 + 3 * b + c:13 + 3 * b + c], vv[:Pn])

                    # scratch writes: scr32 rows 0..4 = ux1p, uy1, ux2, uy2, a2
                    nc.sync.dma_start(scr32.ap()[b, 0, lo:hi], ux1p[:Pn])
                    nc.sync.dma_start(scr32.ap()[b, 1, lo:hi], xyxy[:Pn, 1:2])
                    nc.sync.dma_start(scr32.ap()[b, 2, lo:hi], xyxy[:Pn, 2:3])
                    nc.sync.dma_start(scr32.ap()[b, 3, lo:hi], xyxy[:Pn, 3:4])
                    nc.sync.dma_start(scr32.ap()[b, 4, lo:hi], a2[:Pn])
                    # y floors as fp16
                    yf16 = small.tile([128, 2], f16, tag="yf16")
                    nc.vector.tensor_copy(yf16[:Pn], flr[:Pn, 1:4:2])
                    nc.sync.dma_start(scr16a.ap()[b, lo:hi], yf16[:Pn, 0:1])
                    nc.sync.dma_start(scr16b.ap()[b, lo:hi], yf16[:Pn, 1:2])

            # ================= Stage B: A^T build + D tiles + matmuls =========
            AT = {}
            Dt = {}
            for b in range(BPC if "B" in STAGES else 0):
                y1bc = bcp.tile([128, N], f16, tag="y1bc")
                nc.sync.dma_start(y1bc[:], scr16a.ap()[b:b + 1, :].broadcast_to([128, N]))
                y2bc = bcp.tile([128, N], f16, tag="y2bc")
                nc.sync.dma_start(y2bc[:], scr16b.ap()[b:b + 1, :].broadcast_to([128, N]))
                for t in range(XT):
                    p1 = gp.tile([128, N], f16, tag="p1")
                    nc.vector.tensor_scalar(p1[:], y1bc[:], ioy[:, t:t + 1], None, Alu.is_le)
                    p2 = gp.tile([128, N], f16, tag="p2")
                    nc.vector.tensor_scalar(p2[:], y2bc[:], ioy[:, t:t + 1], None, Alu.is_le)
                    at = atp.tile([128, N], f16, tag=f"at{b}{t}")
                    nc.vector.tensor_sub(at[:], p1[:], p2[:])
                    AT[(b, t)] = at

                for t in range(XT):
                    c1 = chp.tile([128, W], f32, tag="c1")
                    nc.sync.dma_start(c1[:], seg.ap()[b, 0, 128 * t:128 * (t + 1), :])
                    c2 = chp.tile([128, W], f32, tag="c2")
                    nc.sync.dma_start(c2[:], seg.ap()[b, 1, 128 * t:128 * (t + 1), :])
                    d = dp.tile([128, W], f16, tag=f"d{b}{t}")
                    if t % 2 == 0:
                        nc.vector.tensor_sub(d[:], c2[:], c1[:])
                    else:
                        nc.gpsimd.tensor_sub(d[:], c2[:], c1[:])
                    Dt[(b, t)] = d

            # ================= Stage C: per n-chunk matmul + x-mask ==========
            for b in range(BPC if "C" in STAGES else 0):
                for c, (lo, hi) in enumerate(NCH):
                    Pn = hi - lo
                    flr, _ = xf_keep[(b, c)]
                    conf = conf_keep[(b, c)]
                    M0 = psp.tile([128, 384], f32, tag="M0")
                    M1 = psp.tile([128, 384], f32, tag="M1")
                    for xh, M in ((0, M0), (1, M1)):
                        for t in range(XT):
                            nc.tensor.matmul(
                                M[:Pn], AT[(b, t)][:, lo:hi],
                                Dt[(b, t)][:, 384 * xh:384 * (xh + 1)],
                                start=(t == 0), stop=(t == XT - 1))

                    g1 = gp.tile([128, W], f16, tag="g1")
                    nc.vector.tensor_scalar(g1[:Pn], iox16[:Pn], flr[:Pn, 0:1], None, Alu.is_ge)
                    g2 = gp.tile([128, W], f16, tag="g2")
                    nc.vector.tensor_scalar(g2[:Pn], iox16[:Pn], flr[:Pn, 2:3], None, Alu.is_ge)
                    msk = gp.tile([128, W], f16, tag="msk")
                    nc.vector.tensor_sub(msk[:Pn], g1[:Pn], g2[:Pn])

                    import os as _os
                    if _os.environ.get("KNOTTR"):
                        scr = gp.tile([128, 384], f32, tag="scr")
                        nc.vector.tensor_mul(scr[:Pn], M0[:Pn], msk[:Pn, 0:384])
                        sa = small.tile([128, 1], f32, tag="sa")
                        nc.vector.tensor_reduce(sa[:Pn], scr[:Pn], axis=AX, op=Alu.add)
                        scr2 = gp.tile([128, 384], f32, tag="scr2")
                        nc.vector.tensor_mul(scr2[:Pn], M1[:Pn], msk[:Pn, 384:768])
                        sb0 = small.tile([128, 1], f32, tag="sb0")
                        nc.vector.tensor_reduce(sb0[:Pn], scr2[:Pn], axis=AX, op=Alu.add)
                        sb = small.tile([128, 1], f32, tag="sb")
                        nc.vector.tensor_add(sb[:Pn], sa[:Pn], sb0[:Pn])
                    else:
                        scr = gp.tile([128, 384], f32, tag="scr")
                        sa = small.tile([128, 1], f32, tag="sa")
                        nc.vector.tensor_tensor_reduce(
                            scr[:Pn], M0[:Pn], msk[:Pn, 0:384], 1.0, 0.0,
                            Alu.mult, Alu.add, accum_out=sa[:Pn])
                        scr2 = gp.tile([128, 384], f32, tag="scr2")
                        sb = small.tile([128, 1], f32, tag="sb")
                        nc.vector.tensor_tensor_reduce(
                            scr2[:Pn], M1[:Pn], msk[:Pn, 384:768], 1.0, sa[:Pn],
                            Alu.mult, Alu.add, accum_out=sb[:Pn])

                    rc = small.tile([128, 1], f32, tag="rc")
                    nc.vector.tensor_sub(rc[:Pn], flr[:Pn, 3:4], flr[:Pn, 1:2])
                    cc = small.tile([128, 1], f32, tag="cc")
                    nc.vector.tensor_sub(cc[:Pn], flr[:Pn, 2:3], flr[:Pn, 0:1])
                    prod = small.tile([128, 1], f32, tag="prod")
                    nc.vector.tensor_mul(prod[:Pn], rc[:Pn], cc[:Pn])
                    area = small.tile([128, 1], f32, tag="area")
                    nc.vector.tensor_scalar(area[:Pn], prod[:Pn], 1.0, None, Alu.max)
                    rar = small.tile([128, 1], f32, tag="rar")
                    recip(rar[:Pn], area[:Pn], Pn, "ra")
                    sdiv = small.tile([128, 1], f32, tag="sdiv")
                    nc.vector.tensor_mul(sdiv[:Pn], sb[:Pn], rar[:Pn])
                    confv = small.tile([128, 1], f32, tag="confv")
                    nc.vector.scalar_tensor_tensor(
                        confv[:Pn], conf[:Pn], CONF_TH, conf[:Pn], Alu.is_ge, Alu.mult)
                    c1t = small.tile([128, 1], f32, tag="c1t")
                    nc.vector.scalar_tensor_tensor(
                        c1t[:Pn], sdiv[:Pn], 0.0, confv[:Pn], Alu.max, Alu.mult)
                    nc.vector.scalar_tensor_tensor(
                        redcol[:Pn, 3 * b + c:3 * b + c + 1], prod[:Pn], 0.0, c1t[:Pn],
                        Alu.is_gt, Alu.mult)

            # ================= Stage D: IoU + plate/ocr per batch ============
            for b in range(BPC if "D" in STAGES else 0):
                pb = small.tile([128, 4], f32, tag="pb")
                nc.sync.dma_start(pb[:P, :], pbx.ap()[b])
                pcc = small.tile([128, 1], f32, tag="pcc")
                nc.sync.dma_start(pcc[:P], pcf.ap()[b])
                pcrow = small.tile([1, P], f32, tag="pcrow")
                nc.sync.dma_start(pcrow[:], pcf.ap()[b:b + 1, :])
                oc = small.tile([128, 37], f32, tag="oc")
                nc.sync.dma_start(oc[:L, :], ocr.ap()[b])

                phalf = small.tile([128, 2], f32, tag="phalf")
                nc.vector.tensor_scalar(phalf[:P], pb[:P, 2:4], 0.5, None, Alu.mult)
                pxy = small.tile([128, 4], f32, tag="pxy")
                nc.vector.tensor_sub(pxy[:P, 0:2], pb[:P, 0:2], phalf[:P])
                nc.vector.tensor_add(pxy[:P, 2:4], pb[:P, 0:2], phalf[:P])
                pd2 = small.tile([128, 2], f32, tag="pd2")
                nc.vector.tensor_sub(pd2[:P], pxy[:P, 2:4], pxy[:P, 0:2])
                a1e = small.tile([128, 1], f32, tag="a1e")
                nc.vector.tensor_mul(a1e[:P], pd2[:P, 0:1], pd2[:P, 1:2])
                nc.vector.tensor_scalar(a1e[:P], a1e[:P], 1e-8, None, Alu.add)

                vx1b = bcp.tile([128, N], f32, tag="vx1b")
                nc.sync.dma_start(vx1b[:P], scr32.ap()[b, 0:1, :].broadcast_to([P, N]))
                vy1b = bcp.tile([128, N], f32, tag="vy1b")
                nc.sync.dma_start(vy1b[:P], scr32.ap()[b, 1:2, :].broadcast_to([P, N]))
                vx2b = bcp.tile([128, N], f32, tag="vx2b")
                nc.sync.dma_start(vx2b[:P], scr32.ap()[b, 2:3, :].broadcast_to([P, N]))
                vy2b = bcp.tile([128, N], f32, tag="vy2b")
                nc.sync.dma_start(vy2b[:P], scr32.ap()[b, 3:4, :].broadcast_to([P, N]))
                a2b = bcp.tile([128, N], f32, tag="a2b")
                nc.sync.dma_start(a2b[:P], scr32.ap()[b, 4:5, :].broadcast_to([P, N]))

                ltx = gp.tile([128, N], f32, tag="ltx")
                nc.vector.tensor_scalar(ltx[:P], vx1b[:P], pxy[:P, 0:1], None, Alu.max)
                lty = gp.tile([128, N], f32, tag="lty")
                nc.vector.tensor_scalar(lty[:P], vy1b[:P], pxy[:P, 1:2], None, Alu.max)
                wx = gp.tile([128, N], f32, tag="wx")
                nc.vector.scalar_tensor_tensor(
                    wx[:P], vx2b[:P], pxy[:P, 2:3], ltx[:P], Alu.min, Alu.subtract)
                wy = gp.tile([128, N], f32, tag="wy")
                nc.vector.scalar_tensor_tensor(
                    wy[:P], vy2b[:P], pxy[:P, 3:4], lty[:P], Alu.min, Alu.subtract)
                nc.vector.tensor_scalar(wx[:P], wx[:P], 0.0, None, Alu.max)
                nc.vector.tensor_scalar(wy[:P], wy[:P], 0.0, None, Alu.max)
                inter = gp.tile([128, N], f32, tag="inter")
                nc.vector.tensor_mul(inter[:P], wx[:P], wy[:P])
                u1t = gp.tile([128, N], f32, tag="u1t")
                nc.vector.tensor_scalar(u1t[:P], a2b[:P], a1e[:P], None, Alu.add)
                union = gp.tile([128, N], f32, tag="union")
                nc.vector.tensor_sub(union[:P], u1t[:P], inter[:P])
                runi = gp.tile([128, N], f32, tag="runi")
                recip(runi[:P], union[:P], P, "ru")
                iou = gp.tile([128, N], f32, tag="iou")
                nc.vector.tensor_mul(iou[:P], inter[:P], runi[:P])
                mx = small.tile([128, 1], f32, tag="mx")
                nc.vector.tensor_reduce(mx[:P], iou[:P], axis=AX, op=Alu.max)

                r5 = small.tile([128, 1], f32, tag="r5")
                nc.vector.tensor_scalar(r5[:P], mx[:P], -1.0, MIN_IOU, Alu.mult, Alu.add)
                pcv = small.tile([128, 1], f32, tag="pcv")
                nc.vector.scalar_tensor_tensor(
                    pcv[:P], pcc[:P], CONF_TH, pcc[:P], Alu.is_gt, Alu.mult)
                nc.vector.scalar_tensor_tensor(
                    redcol[:P, 6 + 2 * b:7 + 2 * b], r5[:P], 0.0, pcv[:P], Alu.max, Alu.mult)
                nc.vector.tensor_scalar(
                    redcol[:P, 7 + 2 * b:8 + 2 * b], pcc[:P], CONF_TH, None, Alu.is_gt)

                # ocr max per row
                nc.vector.tensor_reduce(redcol[:L, 10 + b:11 + b], oc[:L, :], axis=AX, op=Alu.max)
                # max plate conf (row layout)
                mxp = small.tile([1, 1], f32, tag=f"mxp{b}")
                nc.vector.tensor_reduce(mxp[:], pcrow[:], axis=AX, op=Alu.max)
                conf_keep[("mxp", b)] = mxp

            # ================= Stage E: final reduction ======================
            red_ps = psr.tile([1, RC], f32)
            nc.tensor.matmul(red_ps[:], ones[:], redcol[:], start=True, stop=True)
            red = const.tile([1, RC], f32)
            nc.scalar.copy(red[:], red_ps[:])

            fin = const.tile([1, 12], f32)
            # seg_sum
            nc.vector.tensor_reduce(fin[:, 0:1], red[:, 0:6], axis=AX, op=Alu.add)
            # pd count
            nc.vector.tensor_tensor(fin[:, 1:2], red[:, 7:8], red[:, 9:10], Alu.add)
            # b_valid per batch: pvany * vvany
            for b in range(BPC):
                nc.vector.tensor_reduce(fin[:, 4 + b:5 + b], red[:, 12 + 3 * b:15 + 3 * b],
                                        axis=AX, op=Alu.add)
                nc.vector.tensor_scalar(fin[:, 4 + b:5 + b], fin[:, 4 + b:5 + b], 0.0,
                                        None, Alu.is_gt)
                nc.vector.tensor_scalar(fin[:, 6 + b:7 + b], red[:, 7 + 2 * b:8 + 2 * b],
                                        0.0, None, Alu.is_gt)
                nc.vector.tensor_tensor(fin[:, 8 + b:9 + b], fin[:, 4 + b:5 + b],
                                        fin[:, 6 + b:7 + b], Alu.mult)
                nc.vector.tensor_tensor(fin[:, 8 + b:9 + b], fin[:, 8 + b:9 + b],
                                        red[:, 6 + 2 * b:7 + 2 * b], Alu.mult)
            nc.vector.tensor_tensor(fin[:, 2:3], fin[:, 8:9], fin[:, 9:10], Alu.add)
            # ocr term per batch
            for b in range(BPC if "D" in STAGES else 0):
                mxp = conf_keep[("mxp", b)]
                avg = const.tile([1, 1], f32, tag=f"avg{b}")
                nc.vector.tensor_scalar(avg[:], red[:, 10 + b:11 + b], 1.0 / L, None, Alu.mult)
                dd = const.tile([1, 1], f32, tag=f"dd{b}")
                nc.vector.tensor_sub(dd[:], avg[:], mxp[:])
                gg = const.tile([1, 1], f32, tag=f"gg{b}")
                nc.vector.tensor_scalar(gg[:], avg[:], OCR_TH, None, Alu.is_gt)
                nc.vector.scalar_tensor_tensor(fin[:, 10 + b:11 + b], dd[:], 0.0, gg[:],
                                               Alu.max, Alu.mult)
            nc.vector.tensor_tensor(fin[:, 3:4], fin[:, 10:11], fin[:, 11:12], Alu.add)

            ot = const.tile([1, 4], f32)
            nc.scalar.copy(ot[:, 0:1], fin[:, 0:1])   # seg_sum
            nc.scalar.copy(ot[:, 1:2], fin[:, 2:3])   # pd term (b_valid applied)
            nc.scalar.copy(ot[:, 2:3], fin[:, 1:2])   # pd count
            nc.scalar.copy(ot[:, 3:4], fin[:, 3:4])   # ocr sum
            nc.sync.dma_start(out.ap(), ot[:])

    nc.compile()
    _nc = nc
    return nc


def _shard(inputs):
    in_maps = []
    for c in range(NCORES):
        sl = slice(BPC * c, BPC * (c + 1))
        in_maps.append({
            "seg_ch": np.ascontiguousarray(inputs["seg_masks"][sl, 1:3]),
            "det_boxes": np.ascontiguousarray(inputs["det_boxes"][sl]),
            "det_logits": np.ascontiguousarray(inputs["det_class_logits"][sl]),
            "plate_boxes": np.ascontiguousarray(inputs["plate_boxes"][sl]),
            "plate_conf": np.ascontiguousarray(inputs["plate_confidence"][sl]),
            "ocr": np.ascontiguousarray(inputs["ocr_char_probs"][sl]),
        })
    return in_maps


def _combine(results):
    parts = np.stack([r["partials"][0] for r in results])  # [8, 4]
    seg = parts[:, 0].sum() / np.float32(B * N)
    pd_term = parts[:, 1].sum()
    pd_cnt = parts[:, 2].sum()
    ocr_s = parts[:, 3].sum()
    pd = pd_term / max(pd_cnt, np.float32(1.0))
    op = ocr_s / np.float32(B)
    return np.float32(DS_W * seg + PD_W * pd + OP_W * op)


def kernel(**inputs):
    from concourse.bass_utils import run_bass_kernel_spmd
    nc = _build()
    in_maps = _shard(inputs)
    res = run_bass_kernel_spmd(nc, in_maps, core_ids=list(range(NCORES)))
    return _combine(res.results)
